# revision 2
# baseline (speedup 1.0000x reference)
import numpy as np

SR, SEG, NH, BASE_F = 48000, 960, 8, 220.0
N, C, Lf = 32, 256, 250
Lw = Lf * SEG
NCORES = 8
NPC = N // NCORES   # 4 samples per core
MAGIC = 12582912.0  # 1.5*2^23: fl(x+MAGIC)-MAGIC == round(x) for |x|<2^22

_cache = {}


def _consts():
    s = np.arange(SEG, dtype=np.float64)
    delta = (s + 0.5) / SEG - 0.5
    lo = s < SEG // 2
    a_s = np.where(lo, -delta, 0.0)
    b_s = np.where(lo, 1 + delta, 1 - delta)
    d_s = np.where(lo, 0.0, delta)
    A = np.cumsum(a_s) / SR
    B = np.cumsum(b_s) / SR
    D = np.cumsum(d_s) / SR
    rt = np.stack([A, B, D, np.ones(SEG)]).astype(np.float32)      # [4,960]
    # negated: kernel computes sin(2pi*(round(u)-u)) = -sin(2pi*u)
    rm = (-np.stack([a_s, b_s, d_s])).astype(np.float32)           # [3,960]
    csum = (np.array([[120.0], [720.0], [120.0]]) / SR).astype(np.float32)
    mean8 = np.full((8, 1), 1.0 / NH, np.float32)
    return rt, rm, csum, mean8


def _build():
    import concourse.bacc as bacc
    import concourse.mybir as mybir
    import concourse.tile as tile
    from contextlib import ExitStack

    dt = mybir.dt.float32
    AF = mybir.ActivationFunctionType
    AL = mybir.AluOpType
    LN2 = float(np.log(2.0))
    TWO_PI = float(2.0 * np.pi)

    nc = bacc.Bacc("TRN2", target_bir_lowering=False, debug=False)
    x_d = nc.dram_tensor("x", [NPC, C, Lf], dt, kind="ExternalInput")
    phi_d = nc.dram_tensor("phi", [NPC, 1], dt, kind="ExternalInput")
    wT_d = nc.dram_tensor("wT", [C, NH + 1], dt, kind="ExternalInput")
    b9_d = nc.dram_tensor("bias9", [NH + 1, 1], dt, kind="ExternalInput")
    rt_d = nc.dram_tensor("rt", [4, SEG], dt, kind="ExternalInput")
    rm_d = nc.dram_tensor("rm", [3, SEG], dt, kind="ExternalInput")
    cs_d = nc.dram_tensor("csum", [3, 1], dt, kind="ExternalInput")
    m8_d = nc.dram_tensor("mean8", [NH, 1], dt, kind="ExternalInput")
    bo_d = nc.dram_tensor("boct", [1, 1], dt, kind="ExternalInput")
    out_d = nc.dram_tensor("wave", [NPC, Lf, SEG], dt, kind="ExternalOutput")

    with tile.TileContext(nc) as tc, ExitStack() as ctx:
        const = ctx.enter_context(tc.tile_pool(name="const", bufs=1))
        xp = ctx.enter_context(tc.tile_pool(name="xp", bufs=2))
        vp = ctx.enter_context(tc.tile_pool(name="vp", bufs=NPC))
        sm = ctx.enter_context(tc.tile_pool(name="sm", bufs=2))
        stage = ctx.enter_context(tc.tile_pool(name="stage", bufs=1))
        big = ctx.enter_context(tc.tile_pool(name="big", bufs=2))
        wv = ctx.enter_context(tc.tile_pool(name="wv", bufs=3))
        ps_c = ctx.enter_context(tc.tile_pool(name="ps_c", bufs=1, space="PSUM"))
        ps_s = ctx.enter_context(tc.tile_pool(name="ps_s", bufs=2, space="PSUM"))
        ps_u = ctx.enter_context(tc.tile_pool(name="ps_u", bufs=1, space="PSUM"))
        ps_m = ctx.enter_context(tc.tile_pool(name="ps_m", bufs=1, space="PSUM"))

        wa = const.tile([128, NH + 1], dt)
        wb = const.tile([128, NH + 1], dt)
        nc.gpsimd.dma_start(wa[:], wT_d[0:128, :])
        nc.gpsimd.dma_start(wb[:], wT_d[128:256, :])
        b9 = const.tile([NH + 1, 1], dt)
        nc.gpsimd.dma_start(b9[:], b9_d[:])
        rt = const.tile([4, SEG], dt)
        nc.gpsimd.dma_start(rt[:], rt_d[:])
        rm = const.tile([3, SEG], dt)
        nc.gpsimd.dma_start(rm[:], rm_d[:])
        cs = const.tile([3, 1], dt)
        nc.gpsimd.dma_start(cs[:], cs_d[:])
        m8 = const.tile([NH, 1], dt)
        nc.gpsimd.dma_start(m8[:], m8_d[:])
        phi = const.tile([NPC, 1], dt)
        nc.gpsimd.dma_start(phi[:], phi_d[:])
        bo = const.tile([1, 1], dt)
        nc.gpsimd.dma_start(bo[:], bo_d[:])

        s4 = stage.tile([NPC, Lf], dt)
        Vs, Ws = [], []
        for n in range(NPC):
            xa = xp.tile([128, Lf], dt)
            nc.gpsimd.dma_start(xa[:], x_d[n, 0:128, :])
            xb = xp.tile([128, Lf], dt)
            nc.gpsimd.dma_start(xb[:], x_d[n, 128:256, :])
            pc = ps_c.tile([NH, Lf], dt)
            nc.tensor.matmul(pc[:], wa[:, 0:NH], xa[:], start=True, stop=False)
            nc.tensor.matmul(pc[:], wb[:, 0:NH], xb[:], start=False, stop=True)
            po = ps_c.tile([1, Lf], dt)
            nc.tensor.matmul(po[:], wa[:, NH:NH + 1], xa[:], start=True, stop=False)
            nc.tensor.matmul(po[:], wb[:, NH:NH + 1], xb[:], start=False, stop=True)

            V = vp.tile([4, Lf], dt)
            W = vp.tile([3, Lf], dt)
            Vs.append(V)
            Ws.append(W)
            # g = 220*2^oct = exp(ln2*oct + (ln220 + ln2*b_oct))
            gt = sm.tile([1, Lf], dt)
            nc.scalar.activation(gt[:], po[:], AF.Exp,
                                 bias=bo[0:1, 0:1], scale=LN2)
            # place rows via DMA (compute engines need partition-0 starts)
            nc.gpsimd.dma_start(V[1:2, :], gt[:])
            nc.gpsimd.dma_start(V[0:1, 1:Lf], gt[0:1, 0:Lf - 1])
            nc.gpsimd.dma_start(V[0:1, 0:1], gt[0:1, 0:1])
            nc.gpsimd.dma_start(V[2:3, 0:Lf - 1], gt[0:1, 1:Lf])
            nc.gpsimd.dma_start(V[2:3, Lf - 1:Lf], gt[0:1, Lf - 1:Lf])
            # mag rows: exp(min(z + b_mag, 6))
            zmin = sm.tile([NH, Lf], dt)
            nc.vector.tensor_scalar(zmin[:], pc[:], b9[0:NH, 0:1], 6.0,
                                    AL.add, AL.min)
            mag = sm.tile([NH, Lf], dt)
            nc.scalar.activation(mag[:], zmin[:], AF.Exp)
            # mbar into conv psum row 0 (rows already consumed), DMA rows into W
            nc.tensor.matmul(pc[0:1, :], m8[:], mag[:], start=True, stop=True)
            mb = sm.tile([1, Lf], dt)
            nc.scalar.activation(mb[:], pc[0:1, :], AF.Copy)
            nc.gpsimd.dma_start(W[1:2, :], mb[:])
            nc.gpsimd.dma_start(W[0:1, 1:Lf], mb[0:1, 0:Lf - 1])
            nc.gpsimd.dma_start(W[0:1, 0:1], mb[0:1, 0:1])
            nc.gpsimd.dma_start(W[2:3, 0:Lf - 1], mb[0:1, 1:Lf])
            nc.gpsimd.dma_start(W[2:3, Lf - 1:Lf], mb[0:1, Lf - 1:Lf])
            # segment sums S_k = (120*gm1 + 720*g + 120*gp1)/SR, exact on DVE
            c1, c2 = 120.0 / SR, 720.0 / SR
            sst = sm.tile([1, Lf], dt)
            nc.vector.tensor_scalar(sst[0:1, 1:Lf], gt[0:1, 0:Lf - 1], c1, None,
                                    AL.mult)
            nc.vector.tensor_scalar(sst[0:1, 0:1], gt[0:1, 0:1], c1, None, AL.mult)
            nc.vector.scalar_tensor_tensor(sst[:], gt[:], c2, sst[:],
                                           AL.mult, AL.add)
            nc.vector.scalar_tensor_tensor(sst[0:1, 0:Lf - 1], gt[0:1, 1:Lf], c1,
                                           sst[0:1, 0:Lf - 1], AL.mult, AL.add)
            nc.vector.scalar_tensor_tensor(sst[0:1, Lf - 1:Lf], gt[0:1, Lf - 1:Lf],
                                           c1, sst[0:1, Lf - 1:Lf], AL.mult, AL.add)
            nc.gpsimd.dma_start(s4[n:n + 1, :], sst[:])

        # integer-reduced exclusive prefix + phi
        rS = stage.tile([NPC, Lf], dt)
        nc.vector.tensor_scalar(rS[:], s4[:], MAGIC, MAGIC, AL.add, AL.subtract)
        sf = stage.tile([NPC, Lf], dt)
        nc.vector.tensor_tensor(sf[:], s4[:], rS[:], AL.subtract)
        pinc = stage.tile([NPC, Lf], dt)
        nc.vector.tensor_tensor_scan(pinc[:], sf[:], sf[:], 0.0, AL.add, AL.bypass)
        base = stage.tile([NPC, Lf], dt)
        nc.vector.tensor_tensor(base[:], pinc[:], sf[:], AL.subtract)
        nc.vector.tensor_scalar(base[:], base[:], phi[:, 0:1], None, AL.add)
        rB = stage.tile([NPC, Lf], dt)
        nc.vector.tensor_scalar(rB[:], base[:], MAGIC, MAGIC, AL.add, AL.subtract)
        nc.vector.tensor_tensor(base[:], base[:], rB[:], AL.subtract)
        for n in range(NPC):
            nc.gpsimd.dma_start(Vs[n][3:4, :], base[n:n + 1, :])

        HP = Lf // 2  # 125 segments per half
        for n in range(NPC):
            for h in range(2):
                V, W = Vs[n], Ws[n]
                pu = ps_u.tile([HP, 1024], dt)
                nc.tensor.matmul(pu[:, 0:480], V[:, h * HP:(h + 1) * HP],
                                 rt[:, 0:480], start=True, stop=True)
                nc.tensor.matmul(pu[:, 512:992], V[:, h * HP:(h + 1) * HP],
                                 rt[:, 480:960], start=True, stop=True)
                pm = ps_m.tile([HP, 1024], dt)
                nc.tensor.matmul(pm[:, 0:480], W[:, h * HP:(h + 1) * HP],
                                 rm[:, 0:480], start=True, stop=True)
                nc.tensor.matmul(pm[:, 512:992], W[:, h * HP:(h + 1) * HP],
                                 rm[:, 480:960], start=True, stop=True)
                uv = pu[:].rearrange("p (b c) -> p b c", b=2)[:, :, 0:480]
                mv = pm[:].rearrange("p (b c) -> p b c", b=2)[:, :, 0:480]
                rp = big.tile([HP, SEG], dt)
                rpv = rp[:].rearrange("p (b c) -> p b c", b=2)
                # rp = fl(u + MAGIC)
                nc.scalar.activation(rpv, uv, AF.Copy, bias=MAGIC)
                v = big.tile([HP, SEG], dt)
                vv = v[:].rearrange("p (b c) -> p b c", b=2)
                # v = (rp - MAGIC) - u = round(u) - u  in [-0.5, 0.5]
                nc.vector.scalar_tensor_tensor(vv, rpv, MAGIC, uv,
                                               AL.subtract, AL.subtract)
                sn = big.tile([HP, SEG], dt)
                nc.scalar.activation(sn[:], v[:], AF.Sin, scale=TWO_PI)
                wave = wv.tile([HP, SEG], dt)
                wvv = wave[:].rearrange("p (b c) -> p b c", b=2)
                nc.vector.tensor_tensor(wvv, mv, sn[:].rearrange(
                    "p (b c) -> p b c", b=2), AL.mult)
                nc.gpsimd.dma_start(out_d[n, h * HP:(h + 1) * HP, :], wave[:])

    nc.compile()
    return nc


def _make_in_maps(inputs):
    x, phi, w_mag, b_mag, w_oct, b_oct = (inputs[k] for k in (
        "x", "phi", "w_mag", "b_mag", "w_oct", "b_oct"))
    rt, rm, csum, mean8 = _consts()
    wT = np.concatenate([w_mag[:, :, 0], w_oct[:, :, 0]], axis=0).T.copy()  # [256,9]
    b9 = np.concatenate([b_mag, np.log(220.0) + np.log(2.0) * b_oct]).astype(
        np.float32).reshape(NH + 1, 1)
    in_maps = []
    for c in range(NCORES):
        in_maps.append(dict(
            x=np.ascontiguousarray(x[c * NPC:(c + 1) * NPC]).astype(np.float32),
            phi=np.ascontiguousarray(phi[c * NPC:(c + 1) * NPC, 0]).astype(np.float32),
            wT=wT.astype(np.float32), bias9=b9, rt=rt, rm=rm,
            csum=csum, mean8=mean8,
            boct=np.array([[np.log(220.0) + np.log(2.0) * float(b_oct[0])]],
                          np.float32),
        ))
    return in_maps


def kernel(x, phi, w_mag, b_mag, w_oct, b_oct):
    from concourse.bass_utils import run_bass_kernel_spmd

    if "nc" not in _cache:
        _cache["nc"] = _build()
    nc = _cache["nc"]

    in_maps = _make_in_maps(dict(x=x, phi=phi, w_mag=w_mag, b_mag=b_mag,
                                 w_oct=w_oct, b_oct=b_oct))
    res = run_bass_kernel_spmd(nc, in_maps, core_ids=list(range(NCORES)))
    waves = [res.results[c]["wave"].reshape(NPC, 1, Lw) for c in range(NCORES)]
    return np.concatenate(waves, axis=0)



# revision 11
# speedup vs baseline: 1.3299x; 1.3299x over previous
import math
import numpy as np

SR, SEG, NH, BASE_F = 48000, 960, 8, 220.0
N, C, Lf = 32, 256, 250
Lw = Lf * SEG
NCORES = 8
NPC = N // NCORES   # 4 samples per core
HP = Lf // 2        # 125 segments per half
MAGIC = 12582912.0  # 1.5*2^23: fl(x+MAGIC)-MAGIC == round(x) for |x|<2^22

_cache = {}


def _bf(v):
    import ml_dtypes
    return np.asarray(v, np.float32).astype(ml_dtypes.bfloat16)


def _consts():
    s = np.arange(SEG, dtype=np.float64)
    delta = (s + 0.5) / SEG - 0.5
    lo = s < SEG // 2
    a_s = np.where(lo, -delta, 0.0)
    b_s = np.where(lo, 1 + delta, 1 - delta)
    d_s = np.where(lo, 0.0, delta)
    A = (np.cumsum(a_s) / SR).astype(np.float32)
    D = (np.cumsum(d_s) / SR).astype(np.float32)
    R = (np.cumsum(a_s + b_s + d_s) / SR).astype(np.float32)
    R_hi = _bf(R).astype(np.float32)
    R_lo = R - R_hi
    ucoef = np.stack([R_hi, R_hi, A, D, R_lo, np.ones(SEG, np.float32),
                      np.ones(SEG, np.float32)])
    # v-matmul: u rows, +MAGIC, -MAGIC, then negated u rows -> psum holds
    # round(u) - u directly (systolic accumulation is in row order).
    rhsT = np.zeros((128, SEG), np.float32)
    rhsT[32:39] = ucoef
    rhsT[39] = MAGIC
    rhsT[40] = -MAGIC
    rhsT[41:48] = -ucoef
    rhsT[64] = -a_s
    rhsT[65] = -b_s
    rhsT[66] = -d_s
    return _bf(rhsT)


def _build():
    import concourse.bacc as bacc
    import concourse.mybir as mybir
    import concourse.tile as tile
    from concourse.ap import AP
    from contextlib import ExitStack

    f32 = mybir.dt.float32
    bf16 = mybir.dt.bfloat16
    f16 = mybir.dt.float16
    AF = mybir.ActivationFunctionType
    AL = mybir.AluOpType
    LN2 = float(np.log(2.0))
    TWO_PI = float(2.0 * np.pi)
    c1 = 120.0 / SR
    c2 = 720.0 / SR

    nc = bacc.Bacc("TRN2", target_bir_lowering=False, debug=False)
    x_d = nc.dram_tensor("xr", [128, NPC * 500], f32, kind="ExternalInput")
    wT_d = nc.dram_tensor("wTr", [128, 18], f32, kind="ExternalInput")
    rhs_d = nc.dram_tensor("rhsT", [128, SEG], bf16, kind="ExternalInput")
    b9_d = nc.dram_tensor("b9", [NH, 1], f32, kind="ExternalInput")
    phi_d = nc.dram_tensor("phiT", [1, NPC], f32, kind="ExternalInput")
    bo_d = nc.dram_tensor("boct", [1, 1], f32, kind="ExternalInput")
    m8_d = nc.dram_tensor("m8", [NH, 1], bf16, kind="ExternalInput")
    out_d = nc.dram_tensor("wave", [NPC, Lf, SEG], f16, kind="ExternalOutput")

    with tile.TileContext(nc) as tc, ExitStack() as ctx:
        const = ctx.enter_context(tc.tile_pool(name="const", bufs=1))
        stage = ctx.enter_context(tc.tile_pool(name="stage", bufs=2))
        lhsp = ctx.enter_context(tc.tile_pool(name="lhsp", bufs=2))
        snp = ctx.enter_context(tc.tile_pool(name="snp", bufs=2))
        wvp = ctx.enter_context(tc.tile_pool(name="wvp", bufs=2))
        ps_c = ctx.enter_context(tc.tile_pool(name="ps_c", bufs=1, space="PSUM"))
        ps_v = ctx.enter_context(tc.tile_pool(name="ps_v", bufs=2, space="PSUM"))
        ps_m = ctx.enter_context(tc.tile_pool(name="ps_m", bufs=3, space="PSUM"))

        xall = const.tile([128, NPC * 500], f32)
        nc.sync.dma_start(xall[:], x_d[:])
        wTr = const.tile([128, 18], f32)
        nc.sync.dma_start(wTr[:], wT_d[:])
        rhsT = const.tile([128, SEG], bf16)
        nc.sync.dma_start(rhsT[:], rhs_d[:])
        b9 = const.tile([NH, 1], f32)
        nc.sync.dma_start(b9[:], b9_d[:])
        phiT = const.tile([1, NPC], f32)
        nc.sync.dma_start(phiT[:], phi_d[:])
        bo = const.tile([1, 1], f32)
        nc.sync.dma_start(bo[:], bo_d[:])
        m8 = const.tile([NH, 1], bf16)
        nc.sync.dma_start(m8[:], m8_d[:])
        zbias = const.tile([125, 1], f32)
        nc.gpsimd.memset(zbias[:], 0.0)

        for n in range(NPC):
            # ---- conv: cc rows 0-7 cols 0:250 = mag preact; octave at
            # cols 256:506 of row 0 region (same bank, partition-0 start) ----
            cc = ps_c.tile([8, 512], f32)
            x0 = xall[:, n * 500:n * 500 + 250]
            x1 = xall[:, n * 500 + 250:n * 500 + 500]
            nc.tensor.matmul(cc[:, 0:250], wTr[:, 0:8], x0, start=True, stop=False)
            nc.tensor.matmul(cc[:, 0:250], wTr[:, 9:17], x1, start=False, stop=True)
            nc.tensor.matmul(cc[0:1, 256:506], wTr[:, 8:9], x0,
                             start=True, stop=False)
            nc.tensor.matmul(cc[0:1, 256:506], wTr[:, 17:18], x1,
                             start=False, stop=True)

            # ---- g row (padded) ----
            gpad = stage.tile([1, Lf + 2], f32)
            nc.scalar.activation(gpad[0:1, 1:Lf + 1], cc[0:1, 256:506], AF.Exp,
                                 bias=bo[0:1, 0:1], scale=LN2)
            nc.gpsimd.tensor_copy(gpad[0:1, 0:1], gpad[0:1, 1:2])
            nc.gpsimd.tensor_copy(gpad[0:1, Lf + 1:Lf + 2], gpad[0:1, Lf:Lf + 1])

            # ---- mag rows -> bf16, mean via matmul, padded bf16 row ----
            zmin = stage.tile([NH, Lf], f32)
            nc.vector.tensor_scalar(zmin[:], cc[0:8, 0:250], b9[0:NH, 0:1], 6.0,
                                    AL.add, AL.min)
            magsb = stage.tile([NH, Lf], bf16)
            nc.scalar.activation(magsb[:], zmin[:], AF.Exp)
            nc.tensor.matmul(cc[0:1, 0:250], m8[:], magsb[:], start=True, stop=True)
            mpad = stage.tile([1, Lf + 2], bf16)
            nc.scalar.activation(mpad[0:1, 1:Lf + 1], cc[0:1, 0:250], AF.Copy)
            nc.gpsimd.tensor_copy(mpad[0:1, 0:1], mpad[0:1, 1:2])
            nc.gpsimd.tensor_copy(mpad[0:1, Lf + 1:Lf + 2], mpad[0:1, Lf:Lf + 1])

            # ---- bf16 staging rows (stride-256 window layout) ----
            stg = stage.tile([1, 2560], bf16)
            nc.gpsimd.tensor_copy(stg[0:1, 0:252], gpad[:])              # g_hi
            nc.gpsimd.tensor_tensor(stg[0:1, 256:508], gpad[:],
                                    stg[0:1, 0:252], AL.subtract)        # g_lo
            nc.gpsimd.tensor_tensor(stg[0:1, 513:763], gpad[0:1, 0:250],
                                    gpad[0:1, 1:251], AL.subtract)       # dm
            nc.gpsimd.tensor_tensor(stg[0:1, 769:1019], gpad[0:1, 2:252],
                                    gpad[0:1, 1:251], AL.subtract)       # dp
            nc.gpsimd.tensor_copy(stg[0:1, 1024:1276], stg[0:1, 0:252])  # g_hi2
            nc.gpsimd.memset(stg[0:1, 1792:2304], 1.0)                   # ones rows

            # ---- segment sums + per-sample prefix (fp32, partition 0) ----
            t1 = stage.tile([1, Lf], f32)
            nc.gpsimd.tensor_tensor(t1[:], gpad[0:1, 0:250], gpad[0:1, 2:252],
                                    AL.add)
            xg = stage.tile([1, Lf], f32)
            nc.gpsimd.tensor_scalar(xg[:], gpad[0:1, 1:251], c2, None, AL.mult)
            srow = stage.tile([1, Lf], f32)
            nc.vector.scalar_tensor_tensor(srow[:], t1[:], c1, xg[:],
                                           AL.mult, AL.add)
            rs_ = stage.tile([1, Lf], f32)
            nc.vector.tensor_scalar(rs_[:], srow[:], MAGIC, MAGIC,
                                    AL.add, AL.subtract)
            sf_ = stage.tile([1, Lf], f32)
            nc.gpsimd.tensor_tensor(sf_[:], srow[:], rs_[:], AL.subtract)
            pinc = stage.tile([1, Lf], f32)
            nc.vector.tensor_tensor_scan(pinc[:], sf_[:], sf_[:], 0.0,
                                         AL.add, AL.bypass)
            b0 = stage.tile([1, Lf], f32)
            nc.gpsimd.tensor_tensor(b0[:], pinc[:], sf_[:], AL.subtract)
            b1 = stage.tile([1, Lf], f32)
            nc.vector.tensor_scalar(b1[:], b0[:], phiT[0:1, n:n + 1], None, AL.add)
            rb_ = stage.tile([1, Lf], f32)
            nc.vector.tensor_scalar(rb_[:], b1[:], MAGIC, MAGIC,
                                    AL.add, AL.subtract)
            b2 = stage.tile([1, Lf], f32)
            nc.vector.scalar_tensor_tensor(b2[:], b1[:], 16.0, rb_[:],
                                           AL.add, AL.subtract)
            nc.gpsimd.tensor_copy(stg[0:1, 1281:1531], b2[:])            # b_hi
            nc.gpsimd.tensor_tensor(stg[0:1, 1537:1787], b2[:],
                                    stg[0:1, 1281:1531], AL.subtract)    # b_lo

            # ---- lhsT placement DMAs ----
            lhs = lhsp.tile([128, Lf], bf16)
            sv = stg[:]
            src9 = AP(tensor=sv.tensor, offset=sv.offset + 1,
                      ap=[[sv.ap[0][0], 1], [256, 9], [1, 250]])
            nc.sync.dma_start(lhs[32:41, 0:250], src9)
            src7 = AP(tensor=sv.tensor, offset=sv.offset + 1,
                      ap=[[sv.ap[0][0], 1], [256, 7], [1, 250]])
            nc.sync.dma_start(lhs[41:48, 0:250], src7)
            mv = mpad[:]
            src_m = AP(tensor=mv.tensor, offset=mv.offset,
                       ap=[[mv.ap[0][0], 1], [1, 3], [1, 250]])
            nc.sync.dma_start(lhs[64:67, 0:250], src_m)

            # ---- v matmuls (round(u)-u directly in psum) + mag + ew ----
            sn = snp.tile([125, 1920], f16)
            wv = wvp.tile([125, 1920], f16)
            for h in range(2):
                col0 = h * 125
                pv = ps_v.tile([125, 1024], f32)
                pms = []
                for cchunk in range(2):
                    s0 = cchunk * 480
                    nc.tensor.matmul(pv[:, cchunk * 512:cchunk * 512 + 480],
                                     lhs[32:48, col0:col0 + 125],
                                     rhsT[32:48, s0:s0 + 480],
                                     start=True, stop=True, tile_position=(32, 0))
                    pm = ps_m.tile([125, 512], f32)
                    nc.tensor.matmul(pm[:, 0:480], lhs[64:67, col0:col0 + 125],
                                     rhsT[64:67, s0:s0 + 480],
                                     start=True, stop=True, tile_position=(64, 0))
                    pms.append(pm)
                pvv = pv[:].rearrange("p (b c) -> p b c", b=2)[:, :, 0:480]
                snv = sn[:, h * 960:(h + 1) * 960].rearrange(
                    "p (b c) -> p b c", b=2)
                nc.scalar.activation(snv, pvv, AF.Sin, bias=zbias[:, 0:1],
                                     scale=TWO_PI)
                for cchunk in range(2):
                    q = h * 2 + cchunk
                    nc.vector.tensor_tensor(wv[:, q * 480:(q + 1) * 480],
                                            pms[cchunk][:, 0:480],
                                            sn[:, q * 480:(q + 1) * 480],
                                            AL.mult)
            nc.sync.dma_start(
                out_d[n].rearrange("(h p) s -> p h s", h=2),
                wv[:].rearrange("p (h s) -> p h s", h=2))

    nc.compile()
    return nc


def _make_in_maps(inputs):
    x, phi, w_mag, b_mag, w_oct, b_oct = (inputs[k] for k in (
        "x", "phi", "w_mag", "b_mag", "w_oct", "b_oct"))
    rhsT = _consts()
    W9 = np.concatenate([w_mag[:, :, 0], w_oct[:, :, 0]], axis=0)  # [9, 256]
    wTr = np.zeros((128, 18), np.float32)
    for cc in range(2):
        wTr[:, cc * 9:cc * 9 + 9] = W9[:, cc * 128:(cc + 1) * 128].T
    b9 = np.asarray(b_mag, np.float32).reshape(NH, 1)
    bo = np.array([[math.log(220.0) + math.log(2.0) * float(b_oct[0])]],
                  np.float32)
    m8 = _bf(np.full((NH, 1), 1.0 / NH, np.float32))
    in_maps = []
    for c in range(NCORES):
        xs = np.ascontiguousarray(x[c * NPC:(c + 1) * NPC]).astype(np.float32)
        xr = xs.reshape(NPC, 2, 128, 250).transpose(2, 0, 1, 3).reshape(128, NPC * 500)
        phis = np.ascontiguousarray(
            phi[c * NPC:(c + 1) * NPC, 0, 0]).astype(np.float32).reshape(1, NPC)
        in_maps.append(dict(
            xr=np.ascontiguousarray(xr), wTr=wTr, rhsT=rhsT, b9=b9,
            phiT=phis, boct=bo, m8=m8,
        ))
    return in_maps


def kernel(x, phi, w_mag, b_mag, w_oct, b_oct):
    from concourse.bass_utils import run_bass_kernel_spmd

    if "nc" not in _cache:
        _cache["nc"] = _build()
    nc = _cache["nc"]

    in_maps = _make_in_maps(dict(x=x, phi=phi, w_mag=w_mag, b_mag=b_mag,
                                 w_oct=w_oct, b_oct=b_oct))
    res = run_bass_kernel_spmd(nc, in_maps, core_ids=list(range(NCORES)))
    waves = [res.results[c]["wave"].astype(np.float32).reshape(NPC, 1, Lw)
             for c in range(NCORES)]
    return np.concatenate(waves, axis=0)


# revision 14
# speedup vs baseline: 1.7848x; 1.3421x over previous
import math
import numpy as np

SR, SEG, NH, BASE_F = 48000, 960, 8, 220.0
N, C, Lf = 32, 256, 250
Lw = Lf * SEG
NCORES = 8
NPC = N // NCORES   # 4 samples per core
HP = Lf // 2
MAGIC = 12582912.0

_cache = {}


def _bf(v):
    import ml_dtypes
    return np.asarray(v, np.float32).astype(ml_dtypes.bfloat16)


def _consts():
    s = np.arange(SEG, dtype=np.float64)
    delta = (s + 0.5) / SEG - 0.5
    lo = s < SEG // 2
    a_s = np.where(lo, -delta, 0.0)
    b_s = np.where(lo, 1 + delta, 1 - delta)
    d_s = np.where(lo, 0.0, delta)
    A = (np.cumsum(a_s) / SR).astype(np.float32)
    D = (np.cumsum(d_s) / SR).astype(np.float32)
    R = (np.cumsum(a_s + b_s + d_s) / SR).astype(np.float32)
    R_hi = _bf(R).astype(np.float32)
    R_lo = R - R_hi
    ucoef = np.stack([R_hi, R_hi, A, D, R_lo, np.ones(SEG, np.float32),
                      np.ones(SEG, np.float32)])
    rhs = np.zeros((128, SEG), np.float32)
    rhs[32:39] = ucoef
    rhs[39] = MAGIC
    rhs[40] = -MAGIC
    rhs[41:48] = -ucoef
    coef = np.stack([a_s, b_s, d_s]).astype(np.float32)  # [3, 960]
    for sh in range(3):
        for h in range(NH):
            rhs[64 + sh * 8 + h] = -coef[sh] / NH
    return _bf(rhs)


def _build():
    import concourse.bacc as bacc
    import concourse.mybir as mybir
    import concourse.tile as tile
    from concourse.ap import AP
    from contextlib import ExitStack

    f32 = mybir.dt.float32
    bf16 = mybir.dt.bfloat16
    f16 = mybir.dt.float16
    AF = mybir.ActivationFunctionType
    AL = mybir.AluOpType
    LN2 = float(np.log(2.0))
    TWO_PI = float(2.0 * np.pi)
    c1 = 120.0 / SR
    c2 = 720.0 / SR

    nc = bacc.Bacc("TRN2", target_bir_lowering=False, debug=False)
    x_d = nc.dram_tensor("xr", [128, NPC * 500], f32, kind="ExternalInput")
    xb_d = nc.dram_tensor("xrb", [128, NPC * 500], bf16, kind="ExternalInput")
    cst_d = nc.dram_tensor("cst", [128, 8], f32, kind="ExternalInput")
    rhs_d = nc.dram_tensor("rhsb", [128, SEG + 16], bf16, kind="ExternalInput")
    out_d = nc.dram_tensor("wave", [NPC, Lf, SEG], f16, kind="ExternalOutput")
    stg_d = nc.dram_tensor("stgd", [NPC, 2560], bf16, kind="Internal")
    mp_d = nc.dram_tensor("mpd", [NH, 1008], bf16, kind="Internal")

    with tile.TileContext(nc) as tc, ExitStack() as ctx:
        const = ctx.enter_context(tc.tile_pool(name="const", bufs=1))
        stage = ctx.enter_context(tc.tile_pool(name="stage", bufs=1))
        snp = ctx.enter_context(tc.tile_pool(name="snp", bufs=2))
        wvp = ctx.enter_context(tc.tile_pool(name="wvp", bufs=2))
        ps_c = ctx.enter_context(tc.tile_pool(name="ps_c", bufs=1, space="PSUM"))
        ps_v = ctx.enter_context(tc.tile_pool(name="ps_v", bufs=2, space="PSUM"))
        ps_m = ctx.enter_context(tc.tile_pool(name="ps_m", bufs=2, space="PSUM"))

        xall = const.tile([128, NPC * 500], f32)
        nc.sync.dma_start(xall[:], x_d[:])
        xbf = const.tile([128, NPC * 500], bf16)
        nc.sync.dma_start(xbf[:], xb_d[:])
        cst = const.tile([128, 8], f32)
        nc.sync.dma_start(cst[:], cst_d[:])
        rhsb = const.tile([128, SEG + 16], bf16)
        nc.sync.dma_start(rhsb[:], rhs_d[:])
        zbias = const.tile([125, 1], f32)
        nc.gpsimd.memset(zbias[:], 0.0)
        lhs = const.tile([128, NPC * 250], bf16)

        # ---- conv for all samples: mag rows 0-7 (bf16), octave row 32 (f32)
        cc = ps_c.tile([128, 1024], f32)
        for co in range(2):
            o0 = co * 512
            r0 = co * 500
            nc.tensor.matmul(cc[0:8, o0:o0 + 500], rhsb[:, 960 + 0:960 + 8],
                             xbf[:, r0:r0 + 500], start=True, stop=False)
            nc.tensor.matmul(cc[0:8, o0:o0 + 500], rhsb[:, 968:976],
                             xbf[:, 1000 + r0:1000 + r0 + 500],
                             start=False, stop=True)
            nc.tensor.matmul(cc[32:33, o0:o0 + 500], cst[:, 0:1],
                             xall[:, r0:r0 + 500], start=True, stop=False,
                             tile_position=(0, 32))
            nc.tensor.matmul(cc[32:33, o0:o0 + 500], cst[:, 1:2],
                             xall[:, 1000 + r0:1000 + r0 + 500],
                             start=False, stop=True, tile_position=(0, 32))

        # ---- g rows for all samples ----
        gtmp = stage.tile([1, 1000], f32)
        gi = AP(tensor=cc[:].tensor, offset=cc[:].offset + 32 * 1024,
                ap=[[1024, 1], [512, 2], [1, 500]])
        go = AP(tensor=gtmp.tensor, offset=gtmp[:].offset,
                ap=[[1000, 1], [500, 2], [1, 500]])
        nc.scalar.activation(go, gi, AF.Exp, bias=cst[0:1, 4:5], scale=LN2)
        gpad4 = stage.tile([NPC, Lf + 2], f32)
        gsrc = AP(tensor=gtmp.tensor, offset=gtmp[:].offset,
                  ap=[[1000, 1], [250, 4], [1, 250]])
        nc.sync.dma_start(gpad4[0:4, 1:251], gsrc)
        nc.vector.tensor_copy(gpad4[:, 0:1], gpad4[:, 1:2])
        nc.vector.tensor_copy(gpad4[:, Lf + 1:Lf + 2], gpad4[:, Lf:Lf + 1])

        # ---- mag: zmin -> exp into 252-packed layout, edge dups ----
        zz = stage.tile([NH, 1008], f32)
        zi = AP(tensor=cc[:].tensor, offset=cc[:].offset,
                ap=[[1024, 8], [512, 2], [250, 2], [1, 250]])
        zo = AP(tensor=zz.tensor, offset=zz[:].offset + 1,
                ap=[[1008, 8], [504, 2], [252, 2], [1, 250]])
        nc.vector.tensor_scalar(zo, zi, cst[0:8, 2:3], 6.0, AL.add, AL.min)
        magp = stage.tile([NH, 1008], bf16)
        mo = AP(tensor=magp.tensor, offset=magp[:].offset + 1,
                ap=[[1008, 8], [504, 2], [252, 2], [1, 250]])
        zo2 = AP(tensor=zz.tensor, offset=zz[:].offset + 1,
                 ap=[[1008, 8], [504, 2], [252, 2], [1, 250]])
        nc.scalar.activation(mo, zo2, AF.Exp)
        eL = AP(tensor=magp.tensor, offset=magp[:].offset,
                ap=[[1008, 8], [252, 4], [1, 1]])
        eLs = AP(tensor=magp.tensor, offset=magp[:].offset + 1,
                 ap=[[1008, 8], [252, 4], [1, 1]])
        nc.gpsimd.tensor_copy(eL, eLs)
        eR = AP(tensor=magp.tensor, offset=magp[:].offset + 251,
                ap=[[1008, 8], [252, 4], [1, 1]])
        eRs = AP(tensor=magp.tensor, offset=magp[:].offset + 250,
                 ap=[[1008, 8], [252, 4], [1, 1]])
        nc.gpsimd.tensor_copy(eR, eRs)

        # ---- bf16 staging rows (stride-256 windows per sample row) ----
        stg4 = stage.tile([NPC, 2560], bf16)
        nc.gpsimd.tensor_copy(stg4[0:4, 0:252], gpad4[:])              # g_hi
        nc.vector.tensor_tensor(stg4[0:4, 256:508], gpad4[:],
                                stg4[0:4, 0:252], AL.subtract)         # g_lo
        nc.gpsimd.tensor_tensor(stg4[0:4, 513:763], gpad4[:, 0:250],
                                gpad4[:, 1:251], AL.subtract)          # dm
        nc.gpsimd.tensor_tensor(stg4[0:4, 769:1019], gpad4[:, 2:252],
                                gpad4[:, 1:251], AL.subtract)          # dp
        nc.gpsimd.tensor_copy(stg4[0:4, 1024:1276], stg4[0:4, 0:252])  # g_hi2
        nc.gpsimd.memset(stg4[0:4, 1792:2304], 1.0)                    # ones

        # ---- segment sums + batched prefix ----
        t1 = stage.tile([NPC, Lf], f32)
        nc.gpsimd.tensor_tensor(t1[:], gpad4[:, 0:250], gpad4[:, 2:252], AL.add)
        xg = stage.tile([NPC, Lf], f32)
        nc.vector.tensor_scalar(xg[:], gpad4[:, 1:251], c2, None, AL.mult)
        srow = stage.tile([NPC, Lf], f32)
        nc.vector.scalar_tensor_tensor(srow[:], t1[:], c1, xg[:],
                                       AL.mult, AL.add)
        rs_ = stage.tile([NPC, Lf], f32)
        nc.vector.tensor_scalar(rs_[:], srow[:], MAGIC, MAGIC,
                                AL.add, AL.subtract)
        sf_ = stage.tile([NPC, Lf], f32)
        nc.gpsimd.tensor_tensor(sf_[:], srow[:], rs_[:], AL.subtract)
        pinc = stage.tile([NPC, Lf], f32)
        nc.vector.tensor_tensor_scan(pinc[:], sf_[:], sf_[:], 0.0,
                                     AL.add, AL.bypass)
        b0 = stage.tile([NPC, Lf], f32)
        nc.gpsimd.tensor_tensor(b0[:], pinc[:], sf_[:], AL.subtract)
        b1 = stage.tile([NPC, Lf], f32)
        nc.vector.tensor_scalar(b1[:], b0[:], cst[0:4, 3:4], None, AL.add)
        rb_ = stage.tile([NPC, Lf], f32)
        nc.vector.tensor_scalar(rb_[:], b1[:], MAGIC, MAGIC,
                                AL.add, AL.subtract)
        b2 = stage.tile([NPC, Lf], f32)
        nc.vector.scalar_tensor_tensor(b2[:], b1[:], 16.0, rb_[:],
                                       AL.add, AL.subtract)
        nc.gpsimd.tensor_copy(stg4[0:4, 1281:1531], b2[:])             # b_hi
        nc.vector.tensor_tensor(stg4[0:4, 1537:1787], b2[:],
                                stg4[0:4, 1281:1531], AL.subtract)     # b_lo

        # ---- bounce staging rows through DRAM, place all lhsT at once ----
        nc.sync.dma_start(stg_d[:], stg4[:])
        nc.sync.dma_start(mp_d[:], magp[:])
        usrc9 = AP(tensor=stg_d[:].tensor, offset=1,
                   ap=[[256, 9], [2560, 4], [1, 250]])
        nc.sync.dma_start(lhs[32:41, :], usrc9)
        usrc7 = AP(tensor=stg_d[:].tensor, offset=1,
                   ap=[[256, 7], [2560, 4], [1, 250]])
        nc.sync.dma_start(lhs[41:48, :], usrc7)
        msrc = AP(tensor=mp_d[:].tensor, offset=0,
                  ap=[[1, 3], [1008, 8], [252, 4], [1, 250]])
        nc.sync.dma_start(lhs[64:88, :], msrc)

        # ---- main loop ----
        for n in range(NPC):
            sn = snp.tile([125, 1920], f16)
            wv = wvp.tile([125, 1920], f16)
            for h in range(2):
                col0 = n * 250 + h * 125
                pv = ps_v.tile([125, 1024], f32)
                pms = []
                for cchunk in range(2):
                    s0 = cchunk * 480
                    nc.tensor.matmul(pv[:, cchunk * 512:cchunk * 512 + 480],
                                     lhs[32:48, col0:col0 + 125],
                                     rhsb[32:48, s0:s0 + 480],
                                     start=True, stop=True, tile_position=(32, 0))
                    pm = ps_m.tile([125, 512], f32)
                    nc.tensor.matmul(pm[:, 0:480], lhs[64:88, col0:col0 + 125],
                                     rhsb[64:88, s0:s0 + 480],
                                     start=True, stop=True, tile_position=(64, 0))
                    pms.append(pm)
                pvv = pv[:].rearrange("p (b c) -> p b c", b=2)[:, :, 0:480]
                snv = sn[:, h * 960:(h + 1) * 960].rearrange(
                    "p (b c) -> p b c", b=2)
                nc.scalar.activation(snv, pvv, AF.Sin, bias=zbias[:, 0:1],
                                     scale=TWO_PI)
                for cchunk in range(2):
                    q = h * 2 + cchunk
                    nc.vector.tensor_tensor(wv[:, q * 480:(q + 1) * 480],
                                            pms[cchunk][:, 0:480],
                                            sn[:, q * 480:(q + 1) * 480],
                                            AL.mult)
            nc.gpsimd.dma_start(
                out_d[n].rearrange("(h p) s -> p h s", h=2),
                wv[:].rearrange("p (h s) -> p h s", h=2))

    nc.compile()
    return nc


def _make_in_maps(inputs):
    x, phi, w_mag, b_mag, w_oct, b_oct = (inputs[k] for k in (
        "x", "phi", "w_mag", "b_mag", "w_oct", "b_oct"))
    rhs_base = _consts()  # [128, 960] bf16
    rhsb = np.zeros((128, SEG + 16), np.float32)
    rhsb[:, 0:SEG] = rhs_base.astype(np.float32)
    wm = w_mag[:, :, 0].astype(np.float32)  # [8, 256]
    for cc in range(2):
        rhsb[:, SEG + cc * 8:SEG + cc * 8 + 8] = wm[:, cc * 128:(cc + 1) * 128].T
    rhsb = _bf(rhsb)
    in_maps = []
    for c in range(NCORES):
        xs = np.ascontiguousarray(x[c * NPC:(c + 1) * NPC]).astype(np.float32)
        # cols: cc*1000 + n*250 + l
        xr = xs.reshape(NPC, 2, 128, 250).transpose(2, 1, 0, 3).reshape(128, 2000)
        xr = np.ascontiguousarray(xr)
        cst = np.zeros((128, 8), np.float32)
        cst[:, 0] = w_oct[0, 0:128, 0]
        cst[:, 1] = w_oct[0, 128:256, 0]
        cst[0:8, 2] = b_mag
        cst[0:4, 3] = phi[c * NPC:(c + 1) * NPC, 0, 0]
        cst[0, 4] = math.log(220.0) + math.log(2.0) * float(b_oct[0])
        in_maps.append(dict(xr=xr, xrb=_bf(xr), cst=cst, rhsb=rhsb))
    return in_maps


def kernel(x, phi, w_mag, b_mag, w_oct, b_oct):
    from concourse.bass_utils import run_bass_kernel_spmd

    if "nc" not in _cache:
        _cache["nc"] = _build()
    nc = _cache["nc"]

    in_maps = _make_in_maps(dict(x=x, phi=phi, w_mag=w_mag, b_mag=b_mag,
                                 w_oct=w_oct, b_oct=b_oct))
    res = run_bass_kernel_spmd(nc, in_maps, core_ids=list(range(NCORES)))
    waves = [res.results[c]["wave"].astype(np.float32).reshape(NPC, 1, Lw)
             for c in range(NCORES)]
    return np.concatenate(waves, axis=0)


# revision 15
# speedup vs baseline: 2.0054x; 1.1235x over previous
import math
import numpy as np

SR, SEG, NH, BASE_F = 48000, 960, 8, 220.0
N, C, Lf = 32, 256, 250
Lw = Lf * SEG
NCORES = 8
NPC = N // NCORES   # 4 samples per core
HP = Lf // 2
MAGIC = 12582912.0

_cache = {}


def _bf(v):
    import ml_dtypes
    return np.asarray(v, np.float32).astype(ml_dtypes.bfloat16)


def _consts():
    s = np.arange(SEG, dtype=np.float64)
    delta = (s + 0.5) / SEG - 0.5
    lo = s < SEG // 2
    a_s = np.where(lo, -delta, 0.0)
    b_s = np.where(lo, 1 + delta, 1 - delta)
    d_s = np.where(lo, 0.0, delta)
    A = (np.cumsum(a_s) / SR).astype(np.float32)
    D = (np.cumsum(d_s) / SR).astype(np.float32)
    R = (np.cumsum(a_s + b_s + d_s) / SR).astype(np.float32)
    R_hi = _bf(R).astype(np.float32)
    R_lo = R - R_hi
    ucoef = np.stack([R_hi, R_hi, A, D, R_lo, np.ones(SEG, np.float32),
                      np.ones(SEG, np.float32)])
    rhs = np.zeros((128, SEG), np.float32)
    rhs[32:39] = ucoef
    rhs[39] = MAGIC
    rhs[40] = -MAGIC
    rhs[41:48] = -ucoef
    coef = np.stack([a_s, b_s, d_s]).astype(np.float32)  # [3, 960]
    for sh in range(3):
        for h in range(NH):
            rhs[64 + sh * 8 + h] = -coef[sh] / NH
    return _bf(rhs)


def _build():
    import concourse.bacc as bacc
    import concourse.mybir as mybir
    import concourse.tile as tile
    from concourse.ap import AP
    from contextlib import ExitStack

    f32 = mybir.dt.float32
    bf16 = mybir.dt.bfloat16
    f16 = mybir.dt.float16
    AF = mybir.ActivationFunctionType
    AL = mybir.AluOpType
    LN2 = float(np.log(2.0))
    TWO_PI = float(2.0 * np.pi)
    c1 = 120.0 / SR
    c2 = 720.0 / SR

    nc = bacc.Bacc("TRN2", target_bir_lowering=False, debug=False)
    x_d = nc.dram_tensor("xr", [128, NPC * 500], f32, kind="ExternalInput")
    xb_d = nc.dram_tensor("xrb", [128, NPC * 500], bf16, kind="ExternalInput")
    cst_d = nc.dram_tensor("cst", [128, 8], f32, kind="ExternalInput")
    rhs_d = nc.dram_tensor("rhsb", [128, SEG + 16], bf16, kind="ExternalInput")
    out_d = nc.dram_tensor("wave", [NPC, Lf, SEG], f16, kind="ExternalOutput")
    on_d = nc.dram_tensor("ones2", [2, NPC * 250], bf16, kind="ExternalInput")
    stg_d = nc.dram_tensor("stgd", [NPC, 1280], bf16, kind="Internal")
    sb_d = nc.dram_tensor("stgb", [NPC, 512], bf16, kind="Internal")
    mp_d = nc.dram_tensor("mpd", [NH, 1008], bf16, kind="Internal")

    with tile.TileContext(nc) as tc, ExitStack() as ctx:
        const = ctx.enter_context(tc.tile_pool(name="const", bufs=1))
        stage = ctx.enter_context(tc.tile_pool(name="stage", bufs=1))
        snp = ctx.enter_context(tc.tile_pool(name="snp", bufs=2))
        wvp = ctx.enter_context(tc.tile_pool(name="wvp", bufs=2))
        ps_c = ctx.enter_context(tc.tile_pool(name="ps_c", bufs=1, space="PSUM"))
        ps_v = ctx.enter_context(tc.tile_pool(name="ps_v", bufs=2, space="PSUM"))
        ps_m = ctx.enter_context(tc.tile_pool(name="ps_m", bufs=2, space="PSUM"))

        xall = const.tile([128, NPC * 500], f32)
        xbf = const.tile([128, NPC * 500], bf16)
        for co in range(2):
            xv = xall[:].rearrange("p (c q) -> p c q", c=2)[:, :, co * 500:(co + 1) * 500]
            xs = x_d[:].rearrange("p (c q) -> p c q", c=2)[:, :, co * 500:(co + 1) * 500]
            nc.sync.dma_start(xv, xs)
            xbv = xbf[:].rearrange("p (c q) -> p c q", c=2)[:, :, co * 500:(co + 1) * 500]
            xbs = xb_d[:].rearrange("p (c q) -> p c q", c=2)[:, :, co * 500:(co + 1) * 500]
            nc.gpsimd.dma_start(xbv, xbs)
        cst = const.tile([128, 8], f32)
        nc.sync.dma_start(cst[:], cst_d[:])
        rhsb = const.tile([128, SEG + 16], bf16)
        nc.sync.dma_start(rhsb[:], rhs_d[:])
        zbias = const.tile([125, 1], f32)
        nc.gpsimd.memset(zbias[:], 0.0)
        lhs = const.tile([128, NPC * 250], bf16)
        nc.gpsimd.dma_start(lhs[39:41, :], on_d[:])

        # ---- conv for all samples: mag rows 0-7 (bf16), octave row 32 (f32)
        cc = ps_c.tile([128, 1024], f32)
        for co in range(2):
            o0 = co * 512
            r0 = co * 500
            nc.tensor.matmul(cc[32:33, o0:o0 + 500], cst[:, 0:1],
                             xall[:, r0:r0 + 500], start=True, stop=False,
                             tile_position=(0, 32))
            nc.tensor.matmul(cc[32:33, o0:o0 + 500], cst[:, 1:2],
                             xall[:, 1000 + r0:1000 + r0 + 500],
                             start=False, stop=True, tile_position=(0, 32))
        for co in range(2):
            o0 = co * 512
            r0 = co * 500
            nc.tensor.matmul(cc[0:8, o0:o0 + 500], rhsb[:, 960 + 0:960 + 8],
                             xbf[:, r0:r0 + 500], start=True, stop=False)
            nc.tensor.matmul(cc[0:8, o0:o0 + 500], rhsb[:, 968:976],
                             xbf[:, 1000 + r0:1000 + r0 + 500],
                             start=False, stop=True)

        # ---- g rows for all samples ----
        gtmp = stage.tile([1, 1000], f32)
        gi = AP(tensor=cc[:].tensor, offset=cc[:].offset + 32 * 1024,
                ap=[[1024, 1], [512, 2], [1, 500]])
        go = AP(tensor=gtmp.tensor, offset=gtmp[:].offset,
                ap=[[1000, 1], [500, 2], [1, 500]])
        nc.scalar.activation(go, gi, AF.Exp, bias=cst[0:1, 4:5], scale=LN2)
        gpad4 = stage.tile([NPC, Lf + 2], f32)
        gsrc = AP(tensor=gtmp.tensor, offset=gtmp[:].offset,
                  ap=[[1000, 1], [250, 4], [1, 250]])
        nc.sync.dma_start(gpad4[0:4, 1:251], gsrc)
        nc.vector.tensor_copy(gpad4[:, 0:1], gpad4[:, 1:2])
        nc.vector.tensor_copy(gpad4[:, Lf + 1:Lf + 2], gpad4[:, Lf:Lf + 1])

        # ---- mag: zmin -> exp into 252-packed layout, edge dups ----
        zz = stage.tile([NH, 1008], f32)
        zi = AP(tensor=cc[:].tensor, offset=cc[:].offset,
                ap=[[1024, 8], [512, 2], [250, 2], [1, 250]])
        zo = AP(tensor=zz.tensor, offset=zz[:].offset + 1,
                ap=[[1008, 8], [504, 2], [252, 2], [1, 250]])
        nc.vector.tensor_scalar(zo, zi, cst[0:8, 2:3], 6.0, AL.add, AL.min)
        magp = stage.tile([NH, 1008], bf16)
        mo = AP(tensor=magp.tensor, offset=magp[:].offset + 1,
                ap=[[1008, 8], [504, 2], [252, 2], [1, 250]])
        zo2 = AP(tensor=zz.tensor, offset=zz[:].offset + 1,
                 ap=[[1008, 8], [504, 2], [252, 2], [1, 250]])
        nc.scalar.activation(mo, zo2, AF.Exp)
        eL = AP(tensor=magp.tensor, offset=magp[:].offset,
                ap=[[1008, 8], [252, 4], [1, 1]])
        eLs = AP(tensor=magp.tensor, offset=magp[:].offset + 1,
                 ap=[[1008, 8], [252, 4], [1, 1]])
        nc.gpsimd.tensor_copy(eL, eLs)
        eR = AP(tensor=magp.tensor, offset=magp[:].offset + 251,
                ap=[[1008, 8], [252, 4], [1, 1]])
        eRs = AP(tensor=magp.tensor, offset=magp[:].offset + 250,
                 ap=[[1008, 8], [252, 4], [1, 1]])
        nc.gpsimd.tensor_copy(eR, eRs)

        # ---- bf16 staging rows (stride-256 windows per sample row) ----
        stg4 = stage.tile([NPC, 1280], bf16)
        nc.vector.tensor_copy(stg4[0:4, 0:252], gpad4[:])              # g_hi
        nc.vector.tensor_tensor(stg4[0:4, 256:508], gpad4[:],
                                stg4[0:4, 0:252], AL.subtract)         # g_lo
        nc.gpsimd.tensor_tensor(stg4[0:4, 513:763], gpad4[:, 0:250],
                                gpad4[:, 1:251], AL.subtract)          # dm
        nc.gpsimd.tensor_tensor(stg4[0:4, 769:1019], gpad4[:, 2:252],
                                gpad4[:, 1:251], AL.subtract)          # dp
        nc.gpsimd.tensor_copy(stg4[0:4, 1024:1276], stg4[0:4, 0:252])  # g_hi2

        # ---- segment sums + batched prefix ----
        t1 = stage.tile([NPC, Lf], f32)
        nc.vector.tensor_tensor(t1[:], gpad4[:, 0:250], gpad4[:, 2:252], AL.add)
        xg = stage.tile([NPC, Lf], f32)
        nc.vector.tensor_scalar(xg[:], gpad4[:, 1:251], c2, None, AL.mult)
        srow = stage.tile([NPC, Lf], f32)
        nc.vector.scalar_tensor_tensor(srow[:], t1[:], c1, xg[:],
                                       AL.mult, AL.add)
        rs_ = stage.tile([NPC, Lf], f32)
        nc.vector.tensor_scalar(rs_[:], srow[:], MAGIC, MAGIC,
                                AL.add, AL.subtract)
        sf_ = stage.tile([NPC, Lf], f32)
        nc.vector.tensor_tensor(sf_[:], srow[:], rs_[:], AL.subtract)
        pinc = stage.tile([NPC, Lf], f32)
        nc.vector.tensor_tensor_scan(pinc[:], sf_[:], sf_[:], 0.0,
                                     AL.add, AL.bypass)
        b0 = stage.tile([NPC, Lf], f32)
        nc.vector.tensor_tensor(b0[:], pinc[:], sf_[:], AL.subtract)
        b1 = stage.tile([NPC, Lf], f32)
        nc.vector.tensor_scalar(b1[:], b0[:], cst[0:4, 3:4], None, AL.add)
        rb_ = stage.tile([NPC, Lf], f32)
        nc.vector.tensor_scalar(rb_[:], b1[:], MAGIC, MAGIC,
                                AL.add, AL.subtract)
        b2 = stage.tile([NPC, Lf], f32)
        nc.vector.scalar_tensor_tensor(b2[:], b1[:], 16.0, rb_[:],
                                       AL.add, AL.subtract)
        stgB = stage.tile([NPC, 512], bf16)
        nc.vector.tensor_copy(stgB[0:4, 0:250], b2[:, 0:250])          # b_hi
        nc.vector.tensor_tensor(stgB[0:4, 256:506], b2[:, 0:250],
                                stgB[0:4, 0:250], AL.subtract)         # b_lo

        # ---- bounce staging rows through DRAM, place lhsT blocks ----
        nc.sync.dma_start(stg_d[:], stg4[:])
        nc.gpsimd.dma_start(mp_d[:], magp[:])
        usrc5 = AP(tensor=stg_d[:].tensor, offset=1,
                   ap=[[256, 5], [1280, 4], [1, 250]])
        nc.sync.dma_start(lhs[32:37, :], usrc5)
        usrc5b = AP(tensor=stg_d[:].tensor, offset=1,
                    ap=[[256, 5], [1280, 4], [1, 250]])
        nc.sync.dma_start(lhs[41:46, :], usrc5b)
        nc.sync.dma_start(sb_d[:], stgB[:])
        bsrc = AP(tensor=sb_d[:].tensor, offset=0,
                  ap=[[256, 2], [512, 4], [1, 250]])
        nc.sync.dma_start(lhs[37:39, :], bsrc)
        bsrc2 = AP(tensor=sb_d[:].tensor, offset=0,
                   ap=[[256, 2], [512, 4], [1, 250]])
        nc.sync.dma_start(lhs[46:48, :], bsrc2)
        msrc = AP(tensor=mp_d[:].tensor, offset=0,
                  ap=[[1, 3], [1008, 8], [252, 4], [1, 250]])
        nc.gpsimd.dma_start(lhs[64:88, :], msrc)

        # ---- main loop ----
        for n in range(NPC):
            sn = snp.tile([125, 1920], f16)
            wv = wvp.tile([125, 1920], f16)
            for h in range(2):
                col0 = n * 250 + h * 125
                pv = ps_v.tile([125, 1024], f32)
                pms = []
                for cchunk in range(2):
                    s0 = cchunk * 480
                    nc.tensor.matmul(pv[:, cchunk * 512:cchunk * 512 + 480],
                                     lhs[32:48, col0:col0 + 125],
                                     rhsb[32:48, s0:s0 + 480],
                                     start=True, stop=True, tile_position=(32, 0))
                    pm = ps_m.tile([125, 512], f32)
                    nc.tensor.matmul(pm[:, 0:480], lhs[64:88, col0:col0 + 125],
                                     rhsb[64:88, s0:s0 + 480],
                                     start=True, stop=True, tile_position=(64, 0))
                    pms.append(pm)
                pvv = pv[:].rearrange("p (b c) -> p b c", b=2)[:, :, 0:480]
                snv = sn[:, h * 960:(h + 1) * 960].rearrange(
                    "p (b c) -> p b c", b=2)
                nc.scalar.activation(snv, pvv, AF.Sin, bias=zbias[:, 0:1],
                                     scale=TWO_PI)
                for cchunk in range(2):
                    q = h * 2 + cchunk
                    nc.vector.tensor_tensor(wv[:, q * 480:(q + 1) * 480],
                                            pms[cchunk][:, 0:480],
                                            sn[:, q * 480:(q + 1) * 480],
                                            AL.mult)
            nc.scalar.dma_start(
                out_d[n].rearrange("(h p) s -> p h s", h=2),
                wv[:].rearrange("p (h s) -> p h s", h=2))

    nc.compile()
    return nc


def _make_in_maps(inputs):
    x, phi, w_mag, b_mag, w_oct, b_oct = (inputs[k] for k in (
        "x", "phi", "w_mag", "b_mag", "w_oct", "b_oct"))
    rhs_base = _consts()  # [128, 960] bf16
    rhsb = np.zeros((128, SEG + 16), np.float32)
    rhsb[:, 0:SEG] = rhs_base.astype(np.float32)
    wm = w_mag[:, :, 0].astype(np.float32)  # [8, 256]
    for cc in range(2):
        rhsb[:, SEG + cc * 8:SEG + cc * 8 + 8] = wm[:, cc * 128:(cc + 1) * 128].T
    rhsb = _bf(rhsb)
    in_maps = []
    for c in range(NCORES):
        xs = np.ascontiguousarray(x[c * NPC:(c + 1) * NPC]).astype(np.float32)
        # cols: cc*1000 + n*250 + l
        xr = xs.reshape(NPC, 2, 128, 250).transpose(2, 1, 0, 3).reshape(128, 2000)
        xr = np.ascontiguousarray(xr)
        cst = np.zeros((128, 8), np.float32)
        cst[:, 0] = w_oct[0, 0:128, 0]
        cst[:, 1] = w_oct[0, 128:256, 0]
        cst[0:8, 2] = b_mag
        cst[0:4, 3] = phi[c * NPC:(c + 1) * NPC, 0, 0]
        cst[0, 4] = math.log(220.0) + math.log(2.0) * float(b_oct[0])
        in_maps.append(dict(xr=xr, xrb=_bf(xr), cst=cst, rhsb=rhsb,
                            ones2=_bf(np.ones((2, NPC * 250), np.float32))))
    return in_maps


def kernel(x, phi, w_mag, b_mag, w_oct, b_oct):
    from concourse.bass_utils import run_bass_kernel_spmd

    if "nc" not in _cache:
        _cache["nc"] = _build()
    nc = _cache["nc"]

    in_maps = _make_in_maps(dict(x=x, phi=phi, w_mag=w_mag, b_mag=b_mag,
                                 w_oct=w_oct, b_oct=b_oct))
    res = run_bass_kernel_spmd(nc, in_maps, core_ids=list(range(NCORES)))
    waves = [res.results[c]["wave"].astype(np.float32).reshape(NPC, 1, Lw)
             for c in range(NCORES)]
    return np.concatenate(waves, axis=0)
